# revision 20
# baseline (speedup 1.0000x reference)
"""AttnBlock3D Trainium2 kernel (8-core frame-parallel, GN folded into weights).

Math (per reference):
  hn = GroupNorm32(x) * gamma + beta          # stats over (c/32, t, h, w) -> global over frames
  q/k/v = hn @ w{q,k,v} + b{q,k,v}            # per-frame, per-position linear over channels
  attn  = softmax(q @ k.T / sqrt(c))          # per frame, positions hw=4096
  o     = attn @ v @ wp + bp
  out   = x + o

Key restructure vs the two-pass version: GroupNorm is a per-channel affine
hn = sc*x + bc (sc = gamma*rstd, bc = beta - mean*sc), and the projections are
linear in hn, so it folds into the weights:
  q = (diag(sc) wq)^T x + (wq^T bc + bq)
x is cast to fp8 ONCE (raw), and after the stats AllReduce only a tiny fixup
runs: scale weight rows by sc (12 DVE/ACT ops) + bias folds (small PE matmuls).
This removes the normalize pass entirely and lets the collective trigger as
soon as the stats reduction finishes (~2us after the x stream lands), with all
weight loading/casting overlapped into the collective wait.

Distribution: one frame (b*t = 8) per NeuronCore; 4KB stats AllReduce.

On-chip layouts (SBUF partitions x free):
  X2  [c=512 (2x{128,2}), pos=4096] fp8     raw x, DoubleRow pairs
  KT2 [c_out, pos=4096] fp8                 k^T (bias applied)
  V2  [pos (16x{128,2}), c_out=512] fp8     v natural (no bias; folded into bp'')
  per q-block (512 positions), flash-pipelined over k-chunk pairs jj:
    S_j psum [128, 512] -> exp -> P_j fp8 (rotating slots)
    d  psum [1,512] += ones.T @ P_j   (softmax denominators on PE)
    O  psum [128c, 512] += V_j.T @ P_j
    r = recip(bcast(d)); OT = O; out = wp.T @ OT * r + bp'' + x (f32 residual)
"""

import sys

sys.path.insert(0, "/opt/trn_rl_repo")

import numpy as np

import concourse.bacc as bacc
import concourse.bass as bass
import concourse.mybir as mybir
import concourse.tile as tile
from concourse.bass_utils import run_bass_kernel_spmd

N_CORES = 8
C = 512  # channels
S = 4096  # positions per frame (h*w)
G = 32  # groups
CPG = C // G  # 16 channels per group
PCH = C // 128  # 4 channel chunks of 128 partitions
KCH = S // 128  # 32 position chunks of 128
QB = 512  # q-block size
NQB = S // QB  # 8 q blocks
NTOT = CPG * 8 * S  # group-norm element count per group (global over 8 frames)
EPS = 1e-6
SCALE = float(C) ** -0.5

F32 = mybir.dt.float32
BF16 = mybir.dt.bfloat16
FP8 = mybir.dt.float8e4
F32R = mybir.dt.float32r
AF = mybir.ActivationFunctionType
ALU = mybir.AluOpType
AX = mybir.AxisListType
DR = mybir.MatmulPerfMode.DoubleRow

_NC_CACHE = {}


def build_nc():
    nc = bacc.Bacc("TRN2", target_bir_lowering=False, debug=False, num_devices=N_CORES)

    x_in = nc.dram_tensor("x", [C, S], F32, kind="ExternalInput")
    gamma_in = nc.dram_tensor("gamma", [C], F32, kind="ExternalInput")
    beta_in = nc.dram_tensor("beta", [C], F32, kind="ExternalInput")
    w_in = {}
    b_in = {}
    for nm in ("wq", "wk", "wv", "wp"):
        w_in[nm] = nc.dram_tensor(nm, [C, C], F32, kind="ExternalInput")
    for nm in ("bq", "bk", "bv", "bp"):
        b_in[nm] = nc.dram_tensor(nm, [C], F32, kind="ExternalInput")
    out_d = nc.dram_tensor("out", [C, S], F32, kind="ExternalOutput")

    with tile.TileContext(nc) as tc:
        with (
            tc.tile_pool(name="persist", bufs=1) as pp,
            tc.tile_pool(name="psum", bufs=1, space="PSUM") as psp,
            tc.tile_pool(name="dram", bufs=1, space="DRAM") as dram,
        ):
            # ---- persistent SBUF ----
            X2 = [pp.tile([128, 2, S], FP8, name=f"X2_{cc}") for cc in range(2)]
            KT2 = [pp.tile([128, 2, S], FP8, name=f"KT2_{cc}") for cc in range(2)]
            V2 = [pp.tile([128, 2, C], FP8, name=f"V2_{jj}") for jj in range(KCH // 2)]
            # sc-scaled fp8 weight pairs (built post-collective)
            W2s = {
                nm: [pp.tile([128, 2, C], FP8, name=f"{nm}s_{cc}") for cc in range(2)]
                for nm in ("wq", "wk", "wv")
            }
            Wp2 = [pp.tile([128, 2, C], FP8, name=f"wp2_{cc}") for cc in range(2)]
            bq_p = [pp.tile([128, 1], F32, name=f"bqp{p}") for p in range(PCH)]
            bk_p = [pp.tile([128, 1], F32, name=f"bkp{p}") for p in range(PCH)]
            bpp_p = [pp.tile([128, 1], F32, name=f"bppp{p}") for p in range(PCH)]
            ones2 = pp.tile([128, 2, 16], FP8, name="ones2")
            nc.vector.memset(ones2[:], 1.0)

            ones_row_f = pp.tile([1, 128], F32, name="ones_row_f")
            ones_row = pp.tile([1, 128], F32R, name="ones_row")
            nc.vector.memset(ones_row_f[:], 1.0)
            nc.vector.tensor_copy(ones_row[:], ones_row_f[:])

            # xs holds the raw f32 x for the whole kernel: stats source, fp8
            # cast source, and the residual term in the epilogue (no re-read).
            xs_t = [pp.tile([128, S], F32, name=f"xs{p}") for p in range(PCH)]

            # ---- prologue pool (released before attention main loop) ----
            prolog_cm = tc.tile_pool(name="prolog", bufs=1)
            pl = prolog_cm.__enter__()

            # ---- pass 1: stream x (critical path); sums on DVE, sumsq on ACT.
            # Quarter-tile granularity; nothing else competes for HBM or the
            # DVE until the collective fires (weights are deferred).
            NQ4 = 4
            Q4 = S // NQ4
            stats8 = pl.tile([128, 8], F32, name="stats8")
            sum_q = [pl.tile([128, 1], F32, name=f"sumq{i}") for i in range(NQ4 * PCH)]
            ssq_q = [pl.tile([128, 1], F32, name=f"ssqq{i}") for i in range(NQ4 * PCH)]
            # doorbells on sync + gpsimd: the scalar queue must stay free for
            # the Square chain (doorbells stall the engine on ring credits)
            for p in range(PCH):
                for h in range(NQ4):
                    hsl = slice(h * Q4, (h + 1) * Q4)
                    if (NQ4 * p + h) % 2 == 0:
                        nc.sync.dma_start(xs_t[p][:, hsl], x_in[p * 128 : (p + 1) * 128, hsl])
                    else:
                        nc.gpsimd.dma_start(xs_t[p][:, hsl], x_in[p * 128 : (p + 1) * 128, hsl])
            for p in range(PCH):
                for h in range(NQ4):
                    hsl = slice(h * Q4, (h + 1) * Q4)
                    nc.vector.reduce_sum(sum_q[NQ4 * p + h][:], xs_t[p][:, hsl], axis=AX.X)
                    junk = pl.tile([128, Q4], BF16, name="junk", tag="junk", bufs=2)
                    nc.scalar.activation(
                        junk[:], xs_t[p][:, hsl], AF.Square, accum_out=ssq_q[NQ4 * p + h][:]
                    )
                nc.vector.tensor_tensor(
                    stats8[:, p : p + 1], sum_q[NQ4 * p][:], sum_q[NQ4 * p + 1][:], op=ALU.add
                )
                nc.vector.tensor_tensor(
                    stats8[:, 4 + p : 5 + p], ssq_q[NQ4 * p][:], ssq_q[NQ4 * p + 1][:], op=ALU.add
                )
                for h in range(2, NQ4):
                    nc.vector.tensor_tensor(
                        stats8[:, p : p + 1], stats8[:, p : p + 1], sum_q[NQ4 * p + h][:], op=ALU.add
                    )
                    nc.vector.tensor_tensor(
                        stats8[:, 4 + p : 5 + p], stats8[:, 4 + p : 5 + p], ssq_q[NQ4 * p + h][:], op=ALU.add
                    )
            cc_in = dram.tile([128, 8], F32, name="cc_in")
            cc_out = dram.tile([128, 8], F32, name="cc_out", addr_space="Shared")
            # gpsimd queue: otherwise idle, so the collective handoff never
            # waits behind other DMA descriptors
            nc.gpsimd.dma_start(cc_in[:], stats8[:])
            nc.gpsimd.collective_compute(
                "AllReduce",
                ALU.add,
                replica_groups=[list(range(N_CORES))],
                ins=[cc_in.opt()],
                outs=[cc_out.opt()],
            )

            # ==== collective-wait work (stats-independent). Only wk is loaded
            # here; wq/wv/wp stream during the K/V phases to keep HBM clear
            # while straggler cores are still in pass 1. ====

            # x -> fp8 pairs, h-major
            H = S // 2
            for h in range(2):
                hsl = slice(h * H, (h + 1) * H)
                for p in range(PCH):
                    dst = X2[p // 2][:, p % 2, hsl]
                    if p % 2 == 0:
                        nc.vector.tensor_copy(dst, xs_t[p][:, hsl])
                    else:
                        nc.scalar.copy(dst, xs_t[p][:, hsl])

            # wk loads during the wait (needed immediately post-stats): bf16
            # staging (scale source, since f32 staging rotates away) + original
            # fp8 pairs for the DoubleRow bias-fold matmuls. wq/wv/wp stream
            # later, gated post-stats, casting straight from f32 staging.
            Wb = {"wk": [pl.tile([128, C], BF16, name=f"wkb{p}") for p in range(PCH)]}
            W2o = {
                nm: [pl.tile([128, 2, C], FP8, name=f"{nm}o_{cc}") for cc in range(2)]
                for nm in ("wq", "wk", "wv")
            }
            for p in range(PCH):
                wstg = pl.tile([128, C], F32, name="wstg", tag="wstg", bufs=3)
                if p % 2 == 0:
                    nc.sync.dma_start(wstg[:], w_in["wk"][p * 128 : (p + 1) * 128, :])
                else:
                    nc.scalar.dma_start(wstg[:], w_in["wk"][p * 128 : (p + 1) * 128, :])
                if p % 2 == 0:
                    nc.vector.tensor_copy(Wb["wk"][p][:], wstg[:])
                    nc.scalar.copy(W2o["wk"][p // 2][:, p % 2, :], wstg[:])
                else:
                    nc.scalar.copy(Wb["wk"][p][:], wstg[:])
                    nc.vector.tensor_copy(W2o["wk"][p // 2][:, p % 2, :], wstg[:])

            # small loads: one strided DMA per vector (channel c=128p+part ->
            # [128 part, 4 p]); scalar queue to keep sync free for x
            gam4 = pl.tile([128, 4], F32, name="gam4")
            bet4 = pl.tile([128, 4], F32, name="bet4")
            b_st = {}
            nc.sync.dma_start(gam4[:], gamma_in[:].rearrange("(p a) -> a p", p=4, a=128))
            nc.sync.dma_start(bet4[:], beta_in[:].rearrange("(p a) -> a p", p=4, a=128))
            for nm in ("bq", "bk", "bv", "bp"):
                b4 = pl.tile([128, 4], F32, name=f"{nm}4")
                nc.sync.dma_start(b4[:], b_in[nm][:].rearrange("(p a) -> a p", p=4, a=128))
                b_st[nm] = b4

            # indicator matrices for group-segment sums / broadcasts
            ind_np = np.zeros((128, 8), np.float32)  # [part, gl] = part//16==gl
            for gl in range(8):
                ind_np[16 * gl : 16 * (gl + 1), gl] = 1.0
            ind_d = nc.inline_tensor(ind_np, name="ind_const")
            indt_d = nc.inline_tensor(np.ascontiguousarray(ind_np.T), name="indt_const")
            IND = pl.tile([128, 8], F32, name="IND")
            INDT = pl.tile([8, 128], F32, name="INDT")
            nc.sync.dma_start(IND[:], ind_d[:, :])
            nc.sync.dma_start(INDT[:], indt_d[:, :])

            # ==== post-collective: group stats -> sc/bc -> weight fixup ====
            stats_g = pl.tile([128, 8], F32, name="stats_g")
            nc.sync.dma_start(stats_g[:], cc_out[:])
            ps_g = psp.tile([8, 8], F32, name="ps_g", tag="ps_d", bufs=1)
            # out[gl, j] = sum_part IND[part, gl] * stats_g[part, j]
            nc.tensor.matmul(ps_g[:], IND[:], stats_g[:], start=True, stop=True)
            invN = 1.0 / float(NTOT)
            mean8 = pl.tile([8, 4], F32, name="mean8")
            var8 = pl.tile([8, 4], F32, name="var8")
            rstd8 = pl.tile([8, 4], F32, name="rstd8")
            eps8 = pl.tile([8, 1], F32, name="eps8")
            nc.vector.memset(eps8[:], EPS)
            nc.vector.tensor_scalar_mul(mean8[:], ps_g[:, 0:4], invN)
            nc.vector.tensor_scalar_mul(var8[:], ps_g[:, 4:8], invN)
            nc.vector.tensor_tensor(rstd8[:], mean8[:], mean8[:], op=ALU.mult)
            nc.vector.tensor_tensor(var8[:], var8[:], rstd8[:], op=ALU.subtract)
            nc.scalar.activation(var8[:], var8[:], AF.Sqrt, bias=eps8[:])
            nc.vector.reciprocal(rstd8[:], var8[:])
            # pack [rstd | mean], broadcast groups -> 128 partitions via PE
            rm8 = pl.tile([8, 8], F32, name="rm8")
            nc.vector.tensor_copy(rm8[:, 0:4], rstd8[:])
            nc.vector.tensor_copy(rm8[:, 4:8], mean8[:])
            ps_e = psp.tile([128, 8], F32, name="ps_e", tag="ps_d", bufs=1)
            nc.tensor.matmul(ps_e[:], INDT[:], rm8[:], start=True, stop=True)
            # sc = gamma * rstd; bc = beta - mean * sc
            sc4 = pl.tile([128, 4], F32, name="sc4")
            bc4 = pl.tile([128, 4], F32, name="bc4")
            nc.vector.tensor_tensor(sc4[:], gam4[:], ps_e[:, 0:4], op=ALU.mult)
            nc.vector.tensor_tensor(bc4[:], ps_e[:, 4:8], sc4[:], op=ALU.mult)
            nc.vector.tensor_tensor(bc4[:], bet4[:], bc4[:], op=ALU.subtract)
            # bc as fp8 DoubleRow pairs (moving operand of the bias folds)
            bcb2 = [pl.tile([128, 2, 1], FP8, name=f"bcb2{cc}") for cc in range(2)]
            for p in range(PCH):
                nc.vector.tensor_copy(bcb2[p // 2][:, p % 2, :], bc4[:, p : p + 1])

            # W'k = diag(sc) wk as fp8 pairs from the bf16 staging, DVE/ACT split
            for p in range(PCH):
                dst = W2s["wk"][p // 2][:, p % 2, :]
                if p % 2 == 0:
                    nc.vector.tensor_scalar_mul(dst, Wb["wk"][p][:], sc4[:, p : p + 1])
                else:
                    nc.scalar.activation(
                        dst, Wb["wk"][p][:], AF.Identity, scale=sc4[:, p : p + 1]
                    )

            # deferred weight streams: scalar-queue DMAs emitted after the
            # stats-dependent Sqrt, so their doorbells only ring post-collective
            # (keeps HBM clear for straggler cores still in pass 1); casts run
            # straight from the f32 staging during the K/V phases.
            def load_w2(nm):
                for p in range(PCH):
                    wstg = pl.tile([128, C], F32, name="wstg", tag="wstg", bufs=3)
                    nc.scalar.dma_start(wstg[:], w_in[nm][p * 128 : (p + 1) * 128, :])
                    if nm == "wp":
                        if p % 2 == 0:
                            nc.vector.tensor_copy(Wp2[p // 2][:, p % 2, :], wstg[:])
                        else:
                            nc.scalar.copy(Wp2[p // 2][:, p % 2, :], wstg[:])
                    elif p % 2 == 0:
                        nc.vector.tensor_scalar_mul(
                            W2s[nm][p // 2][:, p % 2, :], wstg[:], sc4[:, p : p + 1]
                        )
                        nc.scalar.copy(W2o[nm][p // 2][:, p % 2, :], wstg[:])
                    else:
                        nc.scalar.activation(
                            W2s[nm][p // 2][:, p % 2, :], wstg[:], AF.Identity,
                            scale=sc4[:, p : p + 1],
                        )
                        nc.vector.tensor_copy(W2o[nm][p // 2][:, p % 2, :], wstg[:])

            # bias folds on PE (fp8 DoubleRow, N=1): b' = W^T bc (+ original
            # bias). Emitted piecewise, interleaved into the K^T phase below so
            # they never delay the first K matmuls.
            bvb2 = [pl.tile([128, 2, 1], FP8, name=f"bvb2{cc}") for cc in range(2)]

            def fold_bias(nm, stash, borig):
                for m in range(PCH):
                    msl = slice(m * 128, (m + 1) * 128)
                    ps_b = psp.tile(
                        [128, 1], F32, name="ps_b",
                        tag="ps_d" if m % 2 == 0 else "ps_o0", bufs=1,
                    )
                    for cc in range(2):
                        nc.tensor.matmul(
                            ps_b[:], W2o[nm][cc][:, :, msl], bcb2[cc][:],
                            perf_mode=DR, start=(cc == 0), stop=(cc == 1),
                        )
                    nc.vector.tensor_tensor(
                        stash[m][:], ps_b[:], borig[:, m : m + 1], op=ALU.add
                    )

            bv_f = [pl.tile([128, 1], F32, name=f"bvf{m}") for m in range(PCH)]

            def fold_bp():
                # bp'' = wp^T b'v + bp
                for m in range(PCH):
                    nc.vector.tensor_copy(bvb2[m // 2][:, m % 2, :], bv_f[m][:])
                for m in range(PCH):
                    msl = slice(m * 128, (m + 1) * 128)
                    ps_bp = psp.tile(
                        [128, 1], F32, name="ps_bp",
                        tag="ps_d" if m % 2 == 0 else "ps_o0", bufs=1,
                    )
                    for cc in range(2):
                        nc.tensor.matmul(
                            ps_bp[:], Wp2[cc][:, :, msl], bvb2[cc][:],
                            perf_mode=DR, start=(cc == 0), stop=(cc == 1),
                        )
                    nc.vector.tensor_tensor(
                        bpp_p[m][:], ps_bp[:], b_st["bp"][:, m : m + 1], op=ALU.add
                    )

            fold_bias("wk", bk_p, b_st["bk"])

            # ---- K^T (bias on ACT/DVE split) and V from X2 via DoubleRow ----
            for n in range(NQB):
                nsl = slice(n * QB, (n + 1) * QB)
                for m in range(PCH):
                    ps_k = psp.tile([128, QB], F32, name="ps_k", tag="ps_s", bufs=3)
                    for cc in range(2):
                        nc.tensor.matmul(
                            ps_k[:],
                            W2s["wk"][cc][:, :, m * 128 : (m + 1) * 128],
                            X2[cc][:, :, nsl],
                            perf_mode=DR,
                            start=(cc == 0),
                            stop=(cc == 1),
                        )
                    dst = KT2[m // 2][:, m % 2, nsl]
                    if (n * PCH + m) % 2 == 0:
                        nc.scalar.activation(dst, ps_k[:], AF.Identity, bias=bk_p[m][:])
                    else:
                        nc.vector.tensor_scalar_add(dst, ps_k[:], bk_p[m][:])
                if n == 0:
                    load_w2("wq")
                elif n == 1:
                    load_w2("wv")
                elif n == 2:
                    fold_bias("wq", bq_p, b_st["bq"])
                elif n == 3:
                    load_w2("wp")
                elif n == 4:
                    fold_bias("wv", bv_f, b_st["bv"])
            for j in range(KCH):
                ps_v = psp.tile([128, C], F32, name="ps_v", tag="ps_s", bufs=3)
                for cc in range(2):
                    nc.tensor.matmul(
                        ps_v[:],
                        X2[cc][:, :, j * 128 : (j + 1) * 128],
                        W2s["wv"][cc][:, :, :],
                        perf_mode=DR,
                        start=(cc == 0),
                        stop=(cc == 1),
                    )
                if j % 2 == 0:
                    nc.vector.tensor_copy(V2[j // 2][:, j % 2, :], ps_v[:])
                else:
                    nc.scalar.copy(V2[j // 2][:, j % 2, :], ps_v[:])

            fold_bp()

            prolog_cm.__exit__(None, None, None)

            # ---- main-loop pool ----
            mainloop_cm = tc.tile_pool(name="mainloop", bufs=1)
            ml = mainloop_cm.__enter__()

            # ---- attention main loop over q-blocks ----
            def emit_qt(qb, m, QT):
                ps_q = psp.tile([128, QB], F32, name="ps_q", tag="ps_s", bufs=3)
                for cc in range(2):
                    nc.tensor.matmul(
                        ps_q[:],
                        W2s["wq"][cc][:, :, m * 128 : (m + 1) * 128],
                        X2[cc][:, :, qb * QB : (qb + 1) * QB],
                        perf_mode=DR,
                        start=(cc == 0),
                        stop=(cc == 1),
                    )
                nc.vector.tensor_scalar_add(QT[m // 2][:, m % 2, :], ps_q[:], bq_p[m][:])

            def make_qt():
                return [
                    ml.tile([128, 2, QB], FP8, name=f"QT{cc}", tag=f"QT{cc}", bufs=2)
                    for cc in range(2)
                ]

            QT_cur = make_qt()
            for m in range(PCH):
                emit_qt(0, m, QT_cur)

            def emit_s(j, QT, P2pair):
                """scores S^T[j] via DoubleRow fp8 -> exp -> P2 half."""
                ps_s = psp.tile([128, QB], F32, name="ps_s", tag="ps_s", bufs=3)
                for cc in range(2):
                    nc.tensor.matmul(
                        ps_s[:],
                        KT2[cc][:, :, j * 128 : (j + 1) * 128],
                        QT[cc][:],
                        perf_mode=DR,
                        start=(cc == 0),
                        stop=(cc == 1),
                    )
                nc.scalar.activation(P2pair[:, j % 2, :], ps_s[:], AF.Exp, scale=SCALE)

            NJJ = KCH // 2  # 16 pairs
            for qb in range(NQB):
                q0 = qb * QB
                QT_next = None
                ps_dd = psp.tile([1, QB], F32, name="ps_dd", tag="ps_d", bufs=1)
                ps_o = [
                    psp.tile([128, QB], F32, name=f"ps_o{mc}", tag=f"ps_o{mc}", bufs=1)
                    for mc in range(PCH)
                ]

                def make_pair():
                    return ml.tile([128, 2, QB], FP8, name="P2", tag="P2", bufs=4)

                P2s = [None] * NJJ
                P2s[0] = make_pair()
                emit_s(0, QT_cur, P2s[0])
                emit_s(1, QT_cur, P2s[0])
                P2s[1] = make_pair()
                emit_s(2, QT_cur, P2s[1])
                emit_s(3, QT_cur, P2s[1])
                for jj in range(NJJ):
                    if jj + 2 < NJJ:
                        P2s[jj + 2] = make_pair()
                        emit_s(2 * jj + 4, QT_cur, P2s[jj + 2])
                        emit_s(2 * jj + 5, QT_cur, P2s[jj + 2])
                    nc.tensor.matmul(
                        ps_dd[:],
                        ones2[:, :, 0:1],
                        P2s[jj][:],
                        perf_mode=DR,
                        start=(jj == 0),
                        stop=(jj == NJJ - 1),
                    )
                    for mc in range(PCH):
                        nc.tensor.matmul(
                            ps_o[mc][:],
                            V2[jj][:, :, mc * 128 : (mc + 1) * 128],
                            P2s[jj][:],
                            perf_mode=DR,
                            start=(jj == 0),
                            stop=(jj == NJJ - 1),
                        )
                    P2s[jj] = None
                    # interleave next block's q^T generation into the PV stream
                    if jj % 4 == 1 and qb + 1 < NQB:
                        if QT_next is None:
                            QT_next = make_qt()
                        emit_qt(qb + 1, (jj - 1) // 4, QT_next)

                # psum attention sums -> fp8 OT2 (mc0 on ACT, rest on DVE so
                # OT2[0] completes after ~1 copy-latency and wp m=0 can start);
                # denominators -> r broadcast in parallel: ACT copy psum->sbuf,
                # PE rank-1 f32r broadcast, DVE recip
                OT2 = [
                    ml.tile([128, 2, QB], FP8, name=f"OT2_{cc}", tag=f"OT2_{cc}", bufs=1)
                    for cc in range(2)
                ]
                for mc in range(PCH):
                    if mc == 0:
                        nc.scalar.copy(OT2[mc // 2][:, mc % 2, :], ps_o[mc][:])
                    else:
                        nc.vector.tensor_copy(OT2[mc // 2][:, mc % 2, :], ps_o[mc][:])
                d_sb = ml.tile([1, QB], F32R, name="d_sb", tag="d_sb", bufs=2)
                r_bc = ml.tile([128, QB], F32, name="r_bc", tag="r_bc", bufs=2)
                nc.scalar.copy(d_sb[:], ps_dd[:])

                # project by wp; epilogue: scale by r, add bp'' and residual x.
                # ps_r is emitted after the first wp matmul so the PE rolls
                # straight from PV into wp with no bubble.
                for m in range(PCH):
                    ps_f = psp.tile([128, QB], F32, name="ps_f", tag=f"ps_o{m}", bufs=1)
                    for cc in range(2):
                        nc.tensor.matmul(
                            ps_f[:],
                            Wp2[cc][:, :, m * 128 : (m + 1) * 128],
                            OT2[cc][:],
                            perf_mode=DR,
                            start=(cc == 0),
                            stop=(cc == 1),
                        )
                    if m == 0:
                        ps_r = psp.tile([128, QB], F32, name="ps_r", tag="ps_s", bufs=3)
                        nc.tensor.matmul(ps_r[:], ones_row[:], d_sb[:], start=True, stop=True)
                        nc.vector.reciprocal_approx_fast(r_bc[:], ps_r[:])
                    on_ = ml.tile([128, QB], F32, name="on", tag="on", bufs=4)
                    os_ = ml.tile([128, QB], F32, name="os", tag="os", bufs=4)
                    nc.vector.tensor_tensor(on_[:], ps_f[:], r_bc[:], op=ALU.mult)
                    nc.vector.scalar_tensor_tensor(
                        os_[:], on_[:], bpp_p[m][:], xs_t[m][:, q0 : q0 + QB],
                        op0=ALU.add, op1=ALU.add,
                    )
                    nc.sync.dma_start(
                        out_d[m * 128 : (m + 1) * 128, q0 : q0 + QB], os_[:]
                    )
                if QT_next is not None:
                    QT_cur = QT_next

            mainloop_cm.__exit__(None, None, None)

    nc.compile()
    return nc


def _get_nc():
    if "nc" not in _NC_CACHE:
        _NC_CACHE["nc"] = build_nc()
    return _NC_CACHE["nc"]


def kernel(x, gamma, beta, wq, bq, wk, bk, wv, bv, wp, bp, **_unused):
    x = np.asarray(x, np.float32)
    b, c, t, h, w = x.shape
    assert (b, c, t, h, w) == (1, C, 8, 64, 64)
    nc = _get_nc()

    shared = {
        "gamma": np.ascontiguousarray(np.asarray(gamma, np.float32)),
        "beta": np.ascontiguousarray(np.asarray(beta, np.float32)),
        "wq": np.ascontiguousarray(np.asarray(wq, np.float32)),
        "bq": np.ascontiguousarray(np.asarray(bq, np.float32)),
        "wk": np.ascontiguousarray(np.asarray(wk, np.float32)),
        "bk": np.ascontiguousarray(np.asarray(bk, np.float32)),
        "wv": np.ascontiguousarray(np.asarray(wv, np.float32)),
        "bv": np.ascontiguousarray(np.asarray(bv, np.float32)),
        "wp": np.ascontiguousarray(np.asarray(wp, np.float32)),
        "bp": np.ascontiguousarray(np.asarray(bp, np.float32)),
    }
    in_maps = []
    for ti in range(t):
        frame = np.ascontiguousarray(x[0, :, ti, :, :].reshape(C, S))
        in_maps.append({"x": frame, **shared})

    res = run_bass_kernel_spmd(nc, in_maps, core_ids=list(range(N_CORES)))

    out = np.empty((1, C, t, h, w), np.float32)
    for ti in range(t):
        out[0, :, ti, :, :] = res.results[ti]["out"].reshape(C, h, w)
    return out


# revision 21
# speedup vs baseline: 1.1144x; 1.1144x over previous
"""AttnBlock3D Trainium2 kernel (8-core frame-parallel, GN folded into weights).

Math (per reference):
  hn = GroupNorm32(x) * gamma + beta          # stats over (c/32, t, h, w) -> global over frames
  q/k/v = hn @ w{q,k,v} + b{q,k,v}            # per-frame, per-position linear over channels
  attn  = softmax(q @ k.T / sqrt(c))          # per frame, positions hw=4096
  o     = attn @ v @ wp + bp
  out   = x + o

Key restructure vs the two-pass version: GroupNorm is a per-channel affine
hn = sc*x + bc (sc = gamma*rstd, bc = beta - mean*sc), and the projections are
linear in hn, so it folds into the weights:
  q = (diag(sc) wq)^T x + (wq^T bc + bq)
x is cast to fp8 ONCE (raw), and after the stats AllReduce only a tiny fixup
runs: scale weight rows by sc (12 DVE/ACT ops) + bias folds (small PE matmuls).
This removes the normalize pass entirely and lets the collective trigger as
soon as the stats reduction finishes (~2us after the x stream lands), with all
weight loading/casting overlapped into the collective wait.

Distribution: one frame (b*t = 8) per NeuronCore; 4KB stats AllReduce.

On-chip layouts (SBUF partitions x free):
  X2  [c=512 (2x{128,2}), pos=4096] fp8     raw x, DoubleRow pairs
  KT2 [c_out, pos=4096] fp8                 k^T (bias applied)
  V2  [pos (16x{128,2}), c_out=512] fp8     v natural (no bias; folded into bp'')
  per q-block (512 positions), flash-pipelined over k-chunk pairs jj:
    S_j psum [128, 512] -> exp -> P_j fp8 (rotating slots)
    d  psum [1,512] += ones.T @ P_j   (softmax denominators on PE)
    O  psum [128c, 512] += V_j.T @ P_j
    r = recip(bcast(d)); OT = O; out = wp.T @ OT * r + bp'' + x (f32 residual)
"""

import sys

sys.path.insert(0, "/opt/trn_rl_repo")

import numpy as np

import concourse.bacc as bacc
import concourse.bass as bass
import concourse.mybir as mybir
import concourse.tile as tile
from concourse.bass_utils import run_bass_kernel_spmd

N_CORES = 8
C = 512  # channels
S = 4096  # positions per frame (h*w)
G = 32  # groups
CPG = C // G  # 16 channels per group
PCH = C // 128  # 4 channel chunks of 128 partitions
KCH = S // 128  # 32 position chunks of 128
QB = 512  # q-block size
NQB = S // QB  # 8 q blocks
NTOT = CPG * 8 * S  # group-norm element count per group (global over 8 frames)
EPS = 1e-6
SCALE = float(C) ** -0.5

F32 = mybir.dt.float32
BF16 = mybir.dt.bfloat16
FP8 = mybir.dt.float8e4
F32R = mybir.dt.float32r
AF = mybir.ActivationFunctionType
ALU = mybir.AluOpType
AX = mybir.AxisListType
DR = mybir.MatmulPerfMode.DoubleRow

_NC_CACHE = {}


def build_nc():
    nc = bacc.Bacc("TRN2", target_bir_lowering=False, debug=False, num_devices=N_CORES)

    x_in = nc.dram_tensor("x", [C, S], F32, kind="ExternalInput")
    gamma_in = nc.dram_tensor("gamma", [C], F32, kind="ExternalInput")
    beta_in = nc.dram_tensor("beta", [C], F32, kind="ExternalInput")
    w_in = {}
    b_in = {}
    for nm in ("wq", "wk", "wv", "wp"):
        w_in[nm] = nc.dram_tensor(nm, [C, C], F32, kind="ExternalInput")
    for nm in ("bq", "bk", "bv", "bp"):
        b_in[nm] = nc.dram_tensor(nm, [C], F32, kind="ExternalInput")
    out_d = nc.dram_tensor("out", [C, S], F32, kind="ExternalOutput")

    with tile.TileContext(nc) as tc:
        with (
            tc.tile_pool(name="persist", bufs=1) as pp,
            tc.tile_pool(name="psum", bufs=1, space="PSUM") as psp,
            tc.tile_pool(name="dram", bufs=1, space="DRAM") as dram,
        ):
            # ---- persistent SBUF ----
            X2 = [pp.tile([128, 2, S], FP8, name=f"X2_{cc}") for cc in range(2)]
            KT2 = [pp.tile([128, 2, S], FP8, name=f"KT2_{cc}") for cc in range(2)]
            V2 = [pp.tile([128, 2, C], FP8, name=f"V2_{jj}") for jj in range(KCH // 2)]
            # sc-scaled fp8 weight pairs (built post-collective)
            W2s = {
                nm: [pp.tile([128, 2, C], FP8, name=f"{nm}s_{cc}") for cc in range(2)]
                for nm in ("wq", "wk", "wv")
            }
            Wp2 = [pp.tile([128, 2, C], FP8, name=f"wp2_{cc}") for cc in range(2)]
            bq_p = [pp.tile([128, 1], F32, name=f"bqp{p}") for p in range(PCH)]
            bk_p = [pp.tile([128, 1], F32, name=f"bkp{p}") for p in range(PCH)]
            bpp_p = [pp.tile([128, 1], F32, name=f"bppp{p}") for p in range(PCH)]
            ones2 = pp.tile([128, 2, 16], FP8, name="ones2")
            nc.vector.memset(ones2[:], 1.0)

            ones_row_f = pp.tile([1, 128], F32, name="ones_row_f")
            ones_row = pp.tile([1, 128], F32R, name="ones_row")
            nc.vector.memset(ones_row_f[:], 1.0)
            nc.vector.tensor_copy(ones_row[:], ones_row_f[:])

            # xs holds the raw f32 x for the whole kernel: stats source, fp8
            # cast source, and the residual term in the epilogue (no re-read).
            xs_t = [pp.tile([128, S], F32, name=f"xs{p}") for p in range(PCH)]

            # ---- prologue pool (released before attention main loop) ----
            prolog_cm = tc.tile_pool(name="prolog", bufs=1)
            pl = prolog_cm.__enter__()

            # ---- pass 1: stream x (critical path); sums on DVE, sumsq on ACT.
            # Quarter-tile granularity; nothing else competes for HBM or the
            # DVE until the collective fires (weights are deferred).
            NQ4 = 4
            Q4 = S // NQ4
            stats8 = pl.tile([128, 8], F32, name="stats8")
            sum_q = [pl.tile([128, 1], F32, name=f"sumq{i}") for i in range(NQ4 * PCH)]
            ssq_q = [pl.tile([128, 1], F32, name=f"ssqq{i}") for i in range(NQ4 * PCH)]
            # doorbells on sync + gpsimd: the scalar queue must stay free for
            # the Square chain (doorbells stall the engine on ring credits)
            for p in range(PCH):
                for h in range(NQ4):
                    hsl = slice(h * Q4, (h + 1) * Q4)
                    if (NQ4 * p + h) % 2 == 0:
                        nc.sync.dma_start(xs_t[p][:, hsl], x_in[p * 128 : (p + 1) * 128, hsl])
                    else:
                        nc.gpsimd.dma_start(xs_t[p][:, hsl], x_in[p * 128 : (p + 1) * 128, hsl])
            for p in range(PCH):
                for h in range(NQ4):
                    hsl = slice(h * Q4, (h + 1) * Q4)
                    nc.vector.reduce_sum(sum_q[NQ4 * p + h][:], xs_t[p][:, hsl], axis=AX.X)
                    junk = pl.tile([128, Q4], BF16, name="junk", tag="junk", bufs=2)
                    nc.scalar.activation(
                        junk[:], xs_t[p][:, hsl], AF.Square, accum_out=ssq_q[NQ4 * p + h][:]
                    )
                nc.vector.tensor_tensor(
                    stats8[:, p : p + 1], sum_q[NQ4 * p][:], sum_q[NQ4 * p + 1][:], op=ALU.add
                )
                nc.vector.tensor_tensor(
                    stats8[:, 4 + p : 5 + p], ssq_q[NQ4 * p][:], ssq_q[NQ4 * p + 1][:], op=ALU.add
                )
                for h in range(2, NQ4):
                    nc.vector.tensor_tensor(
                        stats8[:, p : p + 1], stats8[:, p : p + 1], sum_q[NQ4 * p + h][:], op=ALU.add
                    )
                    nc.vector.tensor_tensor(
                        stats8[:, 4 + p : 5 + p], stats8[:, 4 + p : 5 + p], ssq_q[NQ4 * p + h][:], op=ALU.add
                    )
            cc_in = dram.tile([128, 8], F32, name="cc_in")
            cc_out = dram.tile([128, 8], F32, name="cc_out", addr_space="Shared")
            # gpsimd queue: otherwise idle, so the collective handoff never
            # waits behind other DMA descriptors
            nc.gpsimd.dma_start(cc_in[:], stats8[:])
            nc.gpsimd.collective_compute(
                "AllReduce",
                ALU.add,
                replica_groups=[list(range(N_CORES))],
                ins=[cc_in.opt()],
                outs=[cc_out.opt()],
            )

            # ==== collective-wait work (stats-independent). Only wk is loaded
            # here; wq/wv/wp stream during the K/V phases to keep HBM clear
            # while straggler cores are still in pass 1. ====

            # x -> fp8 pairs, h-major
            H = S // 2
            for h in range(2):
                hsl = slice(h * H, (h + 1) * H)
                for p in range(PCH):
                    dst = X2[p // 2][:, p % 2, hsl]
                    if p % 2 == 0:
                        nc.vector.tensor_copy(dst, xs_t[p][:, hsl])
                    else:
                        nc.scalar.copy(dst, xs_t[p][:, hsl])

            # wk loads during the wait (needed immediately post-stats): bf16
            # staging (scale source, since f32 staging rotates away) + original
            # fp8 pairs for the DoubleRow bias-fold matmuls. wq/wv/wp stream
            # later, gated post-stats, casting straight from f32 staging.
            Wb = {"wk": [pl.tile([128, C], BF16, name=f"wkb{p}") for p in range(PCH)]}
            W2o = {
                nm: [pl.tile([128, 2, C], FP8, name=f"{nm}o_{cc}") for cc in range(2)]
                for nm in ("wq", "wk", "wv")
            }
            for p in range(PCH):
                wstg = pl.tile([128, C], F32, name="wstg", tag="wstg", bufs=3)
                if p % 2 == 0:
                    nc.sync.dma_start(wstg[:], w_in["wk"][p * 128 : (p + 1) * 128, :])
                else:
                    nc.scalar.dma_start(wstg[:], w_in["wk"][p * 128 : (p + 1) * 128, :])
                if p % 2 == 0:
                    nc.vector.tensor_copy(Wb["wk"][p][:], wstg[:])
                    nc.scalar.copy(W2o["wk"][p // 2][:, p % 2, :], wstg[:])
                else:
                    nc.scalar.copy(Wb["wk"][p][:], wstg[:])
                    nc.vector.tensor_copy(W2o["wk"][p // 2][:, p % 2, :], wstg[:])

            # small loads: one strided DMA per vector (channel c=128p+part ->
            # [128 part, 4 p]); scalar queue to keep sync free for x
            gam4 = pl.tile([128, 4], F32, name="gam4")
            bet4 = pl.tile([128, 4], F32, name="bet4")
            b_st = {}
            nc.scalar.dma_start(gam4[:], gamma_in[:].rearrange("(p a) -> a p", p=4, a=128))
            nc.scalar.dma_start(bet4[:], beta_in[:].rearrange("(p a) -> a p", p=4, a=128))
            for nm in ("bq", "bk", "bv", "bp"):
                b4 = pl.tile([128, 4], F32, name=f"{nm}4")
                nc.scalar.dma_start(b4[:], b_in[nm][:].rearrange("(p a) -> a p", p=4, a=128))
                b_st[nm] = b4

            # indicator matrices for group-segment sums / broadcasts
            ind_np = np.zeros((128, 8), np.float32)  # [part, gl] = part//16==gl
            for gl in range(8):
                ind_np[16 * gl : 16 * (gl + 1), gl] = 1.0
            ind_d = nc.inline_tensor(ind_np, name="ind_const")
            indt_d = nc.inline_tensor(np.ascontiguousarray(ind_np.T), name="indt_const")
            IND = pl.tile([128, 8], F32, name="IND")
            INDT = pl.tile([8, 128], F32, name="INDT")
            nc.scalar.dma_start(IND[:], ind_d[:, :])
            nc.scalar.dma_start(INDT[:], indt_d[:, :])

            # ==== post-collective: group stats -> sc/bc -> weight fixup ====
            stats_g = pl.tile([128, 8], F32, name="stats_g")
            nc.sync.dma_start(stats_g[:], cc_out[:])
            ps_g = psp.tile([8, 8], F32, name="ps_g", tag="ps_d", bufs=1)
            # out[gl, j] = sum_part IND[part, gl] * stats_g[part, j]
            nc.tensor.matmul(ps_g[:], IND[:], stats_g[:], start=True, stop=True)
            invN = 1.0 / float(NTOT)
            mean8 = pl.tile([8, 4], F32, name="mean8")
            var8 = pl.tile([8, 4], F32, name="var8")
            rstd8 = pl.tile([8, 4], F32, name="rstd8")
            eps8 = pl.tile([8, 1], F32, name="eps8")
            nc.vector.memset(eps8[:], EPS)
            nc.vector.tensor_scalar_mul(mean8[:], ps_g[:, 0:4], invN)
            nc.vector.tensor_scalar_mul(var8[:], ps_g[:, 4:8], invN)
            nc.vector.tensor_tensor(rstd8[:], mean8[:], mean8[:], op=ALU.mult)
            nc.vector.tensor_tensor(var8[:], var8[:], rstd8[:], op=ALU.subtract)
            nc.scalar.activation(var8[:], var8[:], AF.Sqrt, bias=eps8[:])
            nc.vector.reciprocal(rstd8[:], var8[:])
            # pack [rstd | mean], broadcast groups -> 128 partitions via PE
            rm8 = pl.tile([8, 8], F32, name="rm8")
            nc.vector.tensor_copy(rm8[:, 0:4], rstd8[:])
            nc.vector.tensor_copy(rm8[:, 4:8], mean8[:])
            ps_e = psp.tile([128, 8], F32, name="ps_e", tag="ps_d", bufs=1)
            nc.tensor.matmul(ps_e[:], INDT[:], rm8[:], start=True, stop=True)
            # sc = gamma * rstd; bc = beta - mean * sc
            sc4 = pl.tile([128, 4], F32, name="sc4")
            bc4 = pl.tile([128, 4], F32, name="bc4")
            nc.vector.tensor_tensor(sc4[:], gam4[:], ps_e[:, 0:4], op=ALU.mult)
            nc.vector.tensor_tensor(bc4[:], ps_e[:, 4:8], sc4[:], op=ALU.mult)
            nc.vector.tensor_tensor(bc4[:], bet4[:], bc4[:], op=ALU.subtract)
            # bc as fp8 DoubleRow pairs (moving operand of the bias folds)
            bcb2 = [pl.tile([128, 2, 1], FP8, name=f"bcb2{cc}") for cc in range(2)]
            for p in range(PCH):
                nc.vector.tensor_copy(bcb2[p // 2][:, p % 2, :], bc4[:, p : p + 1])

            # W'k = diag(sc) wk as fp8 pairs from the bf16 staging, DVE/ACT split
            for p in range(PCH):
                dst = W2s["wk"][p // 2][:, p % 2, :]
                if p % 2 == 0:
                    nc.vector.tensor_scalar_mul(dst, Wb["wk"][p][:], sc4[:, p : p + 1])
                else:
                    nc.scalar.activation(
                        dst, Wb["wk"][p][:], AF.Identity, scale=sc4[:, p : p + 1]
                    )

            # deferred weight streams: scalar-queue DMAs emitted after the
            # stats-dependent Sqrt, so their doorbells only ring post-collective
            # (keeps HBM clear for straggler cores still in pass 1); casts run
            # straight from the f32 staging during the K/V phases.
            def load_w2(nm):
                for p in range(PCH):
                    wstg = pl.tile([128, C], F32, name="wstg", tag="wstg", bufs=3)
                    nc.scalar.dma_start(wstg[:], w_in[nm][p * 128 : (p + 1) * 128, :])
                    if nm == "wp":
                        if p % 2 == 0:
                            nc.vector.tensor_copy(Wp2[p // 2][:, p % 2, :], wstg[:])
                        else:
                            nc.scalar.copy(Wp2[p // 2][:, p % 2, :], wstg[:])
                    elif p % 2 == 0:
                        nc.vector.tensor_scalar_mul(
                            W2s[nm][p // 2][:, p % 2, :], wstg[:], sc4[:, p : p + 1]
                        )
                        nc.scalar.copy(W2o[nm][p // 2][:, p % 2, :], wstg[:])
                    else:
                        nc.scalar.activation(
                            W2s[nm][p // 2][:, p % 2, :], wstg[:], AF.Identity,
                            scale=sc4[:, p : p + 1],
                        )
                        nc.vector.tensor_copy(W2o[nm][p // 2][:, p % 2, :], wstg[:])

            # bias folds on PE (fp8 DoubleRow, N=1): b' = W^T bc (+ original
            # bias). Emitted piecewise, interleaved into the K^T phase below so
            # they never delay the first K matmuls.
            bvb2 = [pl.tile([128, 2, 1], FP8, name=f"bvb2{cc}") for cc in range(2)]

            def fold_bias(nm, stash, borig):
                for m in range(PCH):
                    msl = slice(m * 128, (m + 1) * 128)
                    ps_b = psp.tile(
                        [128, 1], F32, name="ps_b",
                        tag="ps_d" if m % 2 == 0 else "ps_o0", bufs=1,
                    )
                    for cc in range(2):
                        nc.tensor.matmul(
                            ps_b[:], W2o[nm][cc][:, :, msl], bcb2[cc][:],
                            perf_mode=DR, start=(cc == 0), stop=(cc == 1),
                        )
                    nc.vector.tensor_tensor(
                        stash[m][:], ps_b[:], borig[:, m : m + 1], op=ALU.add
                    )

            bv_f = [pl.tile([128, 1], F32, name=f"bvf{m}") for m in range(PCH)]

            def fold_bp():
                # bp'' = wp^T b'v + bp
                for m in range(PCH):
                    nc.vector.tensor_copy(bvb2[m // 2][:, m % 2, :], bv_f[m][:])
                for m in range(PCH):
                    msl = slice(m * 128, (m + 1) * 128)
                    ps_bp = psp.tile(
                        [128, 1], F32, name="ps_bp",
                        tag="ps_d" if m % 2 == 0 else "ps_o0", bufs=1,
                    )
                    for cc in range(2):
                        nc.tensor.matmul(
                            ps_bp[:], Wp2[cc][:, :, msl], bvb2[cc][:],
                            perf_mode=DR, start=(cc == 0), stop=(cc == 1),
                        )
                    nc.vector.tensor_tensor(
                        bpp_p[m][:], ps_bp[:], b_st["bp"][:, m : m + 1], op=ALU.add
                    )

            fold_bias("wk", bk_p, b_st["bk"])

            # ---- K^T (bias on ACT/DVE split) and V from X2 via DoubleRow ----
            for n in range(NQB):
                nsl = slice(n * QB, (n + 1) * QB)
                for m in range(PCH):
                    ps_k = psp.tile([128, QB], F32, name="ps_k", tag="ps_s", bufs=3)
                    for cc in range(2):
                        nc.tensor.matmul(
                            ps_k[:],
                            W2s["wk"][cc][:, :, m * 128 : (m + 1) * 128],
                            X2[cc][:, :, nsl],
                            perf_mode=DR,
                            start=(cc == 0),
                            stop=(cc == 1),
                        )
                    dst = KT2[m // 2][:, m % 2, nsl]
                    if (n * PCH + m) % 2 == 0:
                        nc.scalar.activation(dst, ps_k[:], AF.Identity, bias=bk_p[m][:])
                    else:
                        nc.vector.tensor_scalar_add(dst, ps_k[:], bk_p[m][:])
                if n == 0:
                    load_w2("wq")
                elif n == 1:
                    load_w2("wv")
                elif n == 2:
                    fold_bias("wq", bq_p, b_st["bq"])
                elif n == 3:
                    load_w2("wp")
                elif n == 4:
                    fold_bias("wv", bv_f, b_st["bv"])
            for j in range(KCH):
                ps_v = psp.tile([128, C], F32, name="ps_v", tag="ps_s", bufs=3)
                for cc in range(2):
                    nc.tensor.matmul(
                        ps_v[:],
                        X2[cc][:, :, j * 128 : (j + 1) * 128],
                        W2s["wv"][cc][:, :, :],
                        perf_mode=DR,
                        start=(cc == 0),
                        stop=(cc == 1),
                    )
                if j % 2 == 0:
                    nc.vector.tensor_copy(V2[j // 2][:, j % 2, :], ps_v[:])
                else:
                    nc.scalar.copy(V2[j // 2][:, j % 2, :], ps_v[:])

            fold_bp()

            prolog_cm.__exit__(None, None, None)

            # ---- main-loop pool ----
            mainloop_cm = tc.tile_pool(name="mainloop", bufs=1)
            ml = mainloop_cm.__enter__()

            # ---- attention main loop over q-blocks ----
            def emit_qt(qb, m, QT):
                ps_q = psp.tile([128, QB], F32, name="ps_q", tag="ps_s", bufs=3)
                for cc in range(2):
                    nc.tensor.matmul(
                        ps_q[:],
                        W2s["wq"][cc][:, :, m * 128 : (m + 1) * 128],
                        X2[cc][:, :, qb * QB : (qb + 1) * QB],
                        perf_mode=DR,
                        start=(cc == 0),
                        stop=(cc == 1),
                    )
                nc.vector.tensor_scalar_add(QT[m // 2][:, m % 2, :], ps_q[:], bq_p[m][:])

            def make_qt():
                return [
                    ml.tile([128, 2, QB], FP8, name=f"QT{cc}", tag=f"QT{cc}", bufs=2)
                    for cc in range(2)
                ]

            QT_cur = make_qt()
            for m in range(PCH):
                emit_qt(0, m, QT_cur)

            def emit_s(j, QT, P2pair):
                """scores S^T[j] via DoubleRow fp8 -> exp -> P2 half."""
                ps_s = psp.tile([128, QB], F32, name="ps_s", tag="ps_s", bufs=3)
                for cc in range(2):
                    nc.tensor.matmul(
                        ps_s[:],
                        KT2[cc][:, :, j * 128 : (j + 1) * 128],
                        QT[cc][:],
                        perf_mode=DR,
                        start=(cc == 0),
                        stop=(cc == 1),
                    )
                nc.scalar.activation(P2pair[:, j % 2, :], ps_s[:], AF.Exp, scale=SCALE)

            NJJ = KCH // 2  # 16 pairs
            for qb in range(NQB):
                q0 = qb * QB
                QT_next = None
                ps_dd = psp.tile([1, QB], F32, name="ps_dd", tag="ps_d", bufs=1)
                ps_o = [
                    psp.tile([128, QB], F32, name=f"ps_o{mc}", tag=f"ps_o{mc}", bufs=1)
                    for mc in range(PCH)
                ]

                def make_pair():
                    return ml.tile([128, 2, QB], FP8, name="P2", tag="P2", bufs=4)

                P2s = [None] * NJJ
                P2s[0] = make_pair()
                emit_s(0, QT_cur, P2s[0])
                emit_s(1, QT_cur, P2s[0])
                P2s[1] = make_pair()
                emit_s(2, QT_cur, P2s[1])
                emit_s(3, QT_cur, P2s[1])
                for jj in range(NJJ):
                    if jj + 2 < NJJ:
                        P2s[jj + 2] = make_pair()
                        emit_s(2 * jj + 4, QT_cur, P2s[jj + 2])
                        emit_s(2 * jj + 5, QT_cur, P2s[jj + 2])
                    nc.tensor.matmul(
                        ps_dd[:],
                        ones2[:, :, 0:1],
                        P2s[jj][:],
                        perf_mode=DR,
                        start=(jj == 0),
                        stop=(jj == NJJ - 1),
                    )
                    for mc in range(PCH):
                        nc.tensor.matmul(
                            ps_o[mc][:],
                            V2[jj][:, :, mc * 128 : (mc + 1) * 128],
                            P2s[jj][:],
                            perf_mode=DR,
                            start=(jj == 0),
                            stop=(jj == NJJ - 1),
                        )
                    P2s[jj] = None
                    # interleave next block's q^T generation into the PV stream
                    if jj % 4 == 1 and qb + 1 < NQB:
                        if QT_next is None:
                            QT_next = make_qt()
                        emit_qt(qb + 1, (jj - 1) // 4, QT_next)

                # psum attention sums -> fp8 OT2 (mc0 on ACT, rest on DVE so
                # OT2[0] completes after ~1 copy-latency and wp m=0 can start);
                # denominators -> r broadcast in parallel: ACT copy psum->sbuf,
                # PE rank-1 f32r broadcast, DVE recip
                OT2 = [
                    ml.tile([128, 2, QB], FP8, name=f"OT2_{cc}", tag=f"OT2_{cc}", bufs=1)
                    for cc in range(2)
                ]
                for mc in range(PCH):
                    if mc == 0:
                        nc.scalar.copy(OT2[mc // 2][:, mc % 2, :], ps_o[mc][:])
                    else:
                        nc.vector.tensor_copy(OT2[mc // 2][:, mc % 2, :], ps_o[mc][:])
                d_sb = ml.tile([1, QB], F32R, name="d_sb", tag="d_sb", bufs=2)
                r_bc = ml.tile([128, QB], F32, name="r_bc", tag="r_bc", bufs=2)
                nc.scalar.copy(d_sb[:], ps_dd[:])

                # project by wp; epilogue: scale by r, add bp'' and residual x.
                # ps_r is emitted after the first wp matmul so the PE rolls
                # straight from PV into wp with no bubble.
                for m in range(PCH):
                    ps_f = psp.tile([128, QB], F32, name="ps_f", tag=f"ps_o{m}", bufs=1)
                    for cc in range(2):
                        nc.tensor.matmul(
                            ps_f[:],
                            Wp2[cc][:, :, m * 128 : (m + 1) * 128],
                            OT2[cc][:],
                            perf_mode=DR,
                            start=(cc == 0),
                            stop=(cc == 1),
                        )
                    if m == 0:
                        ps_r = psp.tile([128, QB], F32, name="ps_r", tag="ps_s", bufs=3)
                        nc.tensor.matmul(ps_r[:], ones_row[:], d_sb[:], start=True, stop=True)
                        nc.vector.reciprocal_approx_fast(r_bc[:], ps_r[:])
                    on_ = ml.tile([128, QB], F32, name="on", tag="on", bufs=4)
                    os_ = ml.tile([128, QB], F32, name="os", tag="os", bufs=4)
                    nc.vector.tensor_tensor(on_[:], ps_f[:], r_bc[:], op=ALU.mult)
                    nc.vector.scalar_tensor_tensor(
                        os_[:], on_[:], bpp_p[m][:], xs_t[m][:, q0 : q0 + QB],
                        op0=ALU.add, op1=ALU.add,
                    )
                    nc.sync.dma_start(
                        out_d[m * 128 : (m + 1) * 128, q0 : q0 + QB], os_[:]
                    )
                if QT_next is not None:
                    QT_cur = QT_next

            mainloop_cm.__exit__(None, None, None)

    nc.compile()
    return nc


def _get_nc():
    if "nc" not in _NC_CACHE:
        _NC_CACHE["nc"] = build_nc()
    return _NC_CACHE["nc"]


def kernel(x, gamma, beta, wq, bq, wk, bk, wv, bv, wp, bp, **_unused):
    x = np.asarray(x, np.float32)
    b, c, t, h, w = x.shape
    assert (b, c, t, h, w) == (1, C, 8, 64, 64)
    nc = _get_nc()

    shared = {
        "gamma": np.ascontiguousarray(np.asarray(gamma, np.float32)),
        "beta": np.ascontiguousarray(np.asarray(beta, np.float32)),
        "wq": np.ascontiguousarray(np.asarray(wq, np.float32)),
        "bq": np.ascontiguousarray(np.asarray(bq, np.float32)),
        "wk": np.ascontiguousarray(np.asarray(wk, np.float32)),
        "bk": np.ascontiguousarray(np.asarray(bk, np.float32)),
        "wv": np.ascontiguousarray(np.asarray(wv, np.float32)),
        "bv": np.ascontiguousarray(np.asarray(bv, np.float32)),
        "wp": np.ascontiguousarray(np.asarray(wp, np.float32)),
        "bp": np.ascontiguousarray(np.asarray(bp, np.float32)),
    }
    in_maps = []
    for ti in range(t):
        frame = np.ascontiguousarray(x[0, :, ti, :, :].reshape(C, S))
        in_maps.append({"x": frame, **shared})

    res = run_bass_kernel_spmd(nc, in_maps, core_ids=list(range(N_CORES)))

    out = np.empty((1, C, t, h, w), np.float32)
    for ti in range(t):
        out[0, :, ti, :, :] = res.results[ti]["out"].reshape(C, h, w)
    return out
